# revision 49
# baseline (speedup 1.0000x reference)
"""QLoRA-style MLP (fake-quant base + LoRA + exact GeLU) on 8 TRN2 cores.

Sharding: token data-parallel (4096 tokens / 8 cores = 512 tokens per core),
weights replicated.  The only cross-core communication is a tiny AllReduce(max)
for the global fake-quant scale of the hidden activation.

Math per layer (matching the jax reference):
    base = fq(x) @ fq(W) + b          fq(t) = clip(round(t/s), -127, 127) * s,
                                      s = max(max|t|, 1e-8) / 127  (global max)
    lora = 2.0 * (x @ A) @ B          (bf16 operands on device)
    out  = base + lora                (layer 1 additionally GeLU'd, exact erf)

v3 design (vs v1): h kept resident in SBUF as fp16 (no HBM spill/reload),
all LoRA matmuls stream bf16 (1 cycle/row on the PE instead of 4 for f32),
inputs packed into 6 DRAM tensors, weights repacked host-side so streaming
DMAs are few and large (16 x 2MB for W_fc, 32 x 1MB for W_proj).

Device mapping (per core, T=512 tokens):
  L1: psum[ff128, T] = sum_ko wfc[mo][k,ff]^T @ qx[k,T]      (bf16 int matmul)
                       + B_fc[16,ff]^T-slice @ xa1[16,T]     (bf16, K=16)
      h[ff,T](fp16) = Gelu(psum * s1 + b_fc); track per-column absmax
  AllReduce(max) -> s_h
  L2: xa2[16,T] = A_proj^T @ h  (64 bf16 matmuls, overlaps the AllReduce)
      qh[ff,T](bf16) = round(h / s_h)   (ACT scale + DVE magic-round)
      psum[tok128, 512] = sum_ko qh[k,tok]^T-tiles @ wproj[no][k,512]
                          + xa2[16,tok]^T-slice @ B_proj[16,512]
      out = psum * s2 + b_proj
"""

import os
import sys

import numpy as np

if "/opt/trn_rl_repo" not in sys.path:
    sys.path.insert(0, "/opt/trn_rl_repo")

import ml_dtypes

# Problem shapes (hardcoded per contract).
B_, S, D, DFF, R = 2, 2048, 2048, 8192, 16
T = B_ * S  # 4096 tokens
NCORES = 8
TC = T // NCORES  # 512 tokens per core
QMAX = np.float32(127.0)
MAGIC = float(np.float32(12582912.0))  # 1.5 * 2**23: fp32 round-half-even trick

KO1 = D // 128  # 16  k-tiles for layer 1
MO1 = DFF // 512  # 16  512-wide ff blocks
M64 = DFF // 128  # 64  128-wide ff blocks
KO2 = DFF // 128  # 64  k-tiles for layer 2
NO2 = D // 512  # 4   512-wide output-col blocks
MT = TC // 128  # 4   token tiles per core
W2CH = 8  # W_proj ko-tiles per streamed chunk
NCH2 = KO2 // W2CH  # 8 chunks per no

_CACHE = {}
LAST_RESULT = None


def _build_nc(n_cores=NCORES, flags=(), loop_k=None):
    """Build + compile the Bass program.

    loop_k: when set, wrap the whole forward pass in a hardware For loop that
    executes it ``loop_k`` times per NEFF launch (used by ``bench`` to measure
    steady-state device throughput without per-dispatch tunnel overhead)."""
    from contextlib import ExitStack

    import concourse.bass as bass  # noqa: F401
    import concourse.mybir as mybir
    import concourse.tile as tile
    from concourse import bacc, bass_isa
    from concourse.bass import ds, ts

    f32 = mybir.dt.float32
    bf16 = mybir.dt.bfloat16
    fp16 = mybir.dt.float16
    AF = mybir.ActivationFunctionType
    ALU = mybir.AluOpType

    nc = bacc.Bacc(None, target_bir_lowering=False, num_devices=n_cores)

    # ---- kernel I/O -------------------------------------------------------
    # blob: all bf16 data in [chunk, 128, 8, 512] granules:
    #   chunks 0-1:  qx^T  (quantized x, int-valued; [128, 16ko, 512])
    #   chunks 2-3:  x^T   (bf16 x for the lora path)
    #   chunks 4-35: W_fc  (chunk 4 + 2*mo + half)
    #   chunks 36-67: W_proj (chunk 36 + 8*no + g)
    #   chunk 68:    A_fc^T ++ A_proj^T tiles ([128, 80, 16], padded)
    #   chunks 69-70: B_fc   ([16, 8192], partitions 0-15 only)
    #   chunk 71:    B_proj ([16, 2048], partitions 0-15 only)
    #   chunks 72-73: fpk hi/lo bf16 split of the f32 pack (reassembled on DVE):
    #     [:, 0:M64] biasfc (col mi), [:, M64:M64+D] biasproj, [:, M64+D:] scal
    #     scal columns: 0: s1 = s_x*s_wfc, 1: c1 = 2/s1, 2: s_wproj, 3: unused
    blob_t = nc.dram_tensor("blob", [74, 128, 8, 512], bf16, kind="ExternalInput")
    out_t = nc.dram_tensor("out", [MT, 128, D], f32, kind="ExternalOutput")
    FW = 5 * 512  # fpk row width (2116 cols padded to 2560)

    SC = M64 + D  # scal column base in fpk
    p1 = "phase2_only" not in flags
    p2 = "phase1_only" not in flags

    with tile.TileContext(nc) as tc:
        with ExitStack() as ctx:
            consts = ctx.enter_context(tc.tile_pool(name="consts", bufs=1))
            dram = ctx.enter_context(tc.tile_pool(name="dram", bufs=1, space="DRAM"))

            # whole-kernel residents
            fpk_sb = consts.tile([128, FW], f32)
            fpk_hi = consts.tile([128, FW], bf16)
            fpk_lo = consts.tile([128, FW], bf16)
            apk_sb = consts.tile([128, KO1 + KO2, R], bf16)
            bpj_sb = consts.tile([R, D], bf16)
            h_sb = consts.tile([128, KO2, TC], fp16)
            xa2_sb = consts.tile([R, TC], bf16)
            maxcol = consts.tile([128, M64], f32)
            ar_in = dram.tile([128, 1], f32)
            ar_out = dram.tile(
                [128, 1], f32, addr_space="Shared" if n_cores > 4 else "Local"
            )
            nc.scalar.dma_start(fpk_hi[:], blob_t[72, :, ds(0, 5), :])
            nc.scalar.dma_start(fpk_lo[:], blob_t[73, :, ds(0, 5), :])
            nc.vector.tensor_tensor(fpk_sb[:], fpk_hi[:], fpk_lo[:], op=ALU.add)
            nc.scalar.dma_start(apk_sb[:, ds(0, 64), :], blob_t[68, :, ds(0, 2), :])
            nc.scalar.dma_start(apk_sb[:, ds(64, 16), :], blob_t[68, :, 2, ds(0, 256)])
            nc.scalar.dma_start(bpj_sb[:], blob_t[71, ds(0, R), ds(0, 4), :])
            scal_sb = fpk_sb[:, ds(SC, 4)]

            if loop_k:
                ctx.enter_context(tc.For_i(0, loop_k, 1))

            # ---- phase 1: h = Gelu(s1 * (qx@qW + lora1/s1) + b_fc) -----------
            with tc.tile_pool(name="ph1c", bufs=1) as ph1c, tc.tile_pool(
                name="wfc", bufs=3
            ) as wp, tc.tile_pool(name="ps1", bufs=2, space="PSUM") as pp:
                xpk_sb = ph1c.tile([128, 2 * KO1, TC], bf16)
                bfc_sb = ph1c.tile([R, DFF], bf16)
                xa1_sb = ph1c.tile([R, TC], bf16)
                if p1:
                    # sync queue: qx chunks interleaved with the first weight
                    # halves so the first matmul isn't stuck behind the whole x
                    # pack; scalar queue: lora consts + unquantized x.
                    w_mo0 = wp.tile([128, KO1, 512], bf16, tag="wfc", name="w_mo")
                    nc.sync.dma_start(xpk_sb[:, ds(0, 4), :], blob_t[0, :, ds(0, 4), :])
                    nc.sync.dma_start(w_mo0[:, ds(0, 8), :], blob_t[4])
                    nc.sync.dma_start(xpk_sb[:, ds(4, 4), :], blob_t[0, :, ds(4, 4), :])
                    nc.sync.dma_start(w_mo0[:, ds(8, 8), :], blob_t[5])
                    nc.sync.dma_start(xpk_sb[:, ds(8, 8), :], blob_t[1])
                    nc.scalar.dma_start(bfc_sb[:, ds(0, 4096)], blob_t[69, ds(0, R)])
                    nc.scalar.dma_start(bfc_sb[:, ds(4096, 4096)], blob_t[70, ds(0, R)])
                    for c in range(2):
                        nc.scalar.dma_start(
                            xpk_sb[:, ds(KO1 + 8 * c, 8), :], blob_t[2 + c]
                        )

                for mo in range(MO1 if p1 else 0):
                    if mo == 0:
                        w_mo = w_mo0
                    else:
                        w_mo = wp.tile([128, KO1, 512], bf16, tag="wfc", name="w_mo")
                        for half in range(2):
                            nc.sync.dma_start(
                                w_mo[:, ds(8 * half, 8), :],
                                blob_t[4 + 2 * mo + half],
                            )
                    pss = [
                        pp.tile([128, TC], f32, tag=f"ps1_{i}", name="ps1t")
                        for i in range(4)
                    ]
                    for ko in range(KO1):
                        for sub in range(4):
                            nc.tensor.matmul(
                                pss[sub][:],
                                w_mo[:, ko, ts(sub, 128)],
                                xpk_sb[:, ko, :],
                                start=(ko == 0),
                                stop=False,
                            )
                    if mo == 0:
                        # xa1 = bf16((x @ A_fc)^T * (2/s1))
                        ps_a1 = pp.tile([128, TC], f32, tag="ps1_0", name="psa1")
                        for ko in range(KO1):
                            nc.tensor.matmul(
                                ps_a1[:R, :],
                                apk_sb[:, ko, :],
                                xpk_sb[:, KO1 + ko, :],
                                start=(ko == 0),
                                stop=(ko == KO1 - 1),
                            )
                        nc.vector.tensor_scalar_mul(
                            xa1_sb[:], ps_a1[:R, :], scal_sb[:R, 1:2]
                        )
                    for sub in range(4):
                        mi = 4 * mo + sub
                        nc.tensor.matmul(
                            pss[sub][:],
                            bfc_sb[:, ts(mi, 128)],
                            xa1_sb[:],
                            start=False,
                            stop=True,
                        )
                        nc.scalar.activation(
                            h_sb[:, mi, :],
                            pss[sub][:],
                            AF.Gelu,
                            bias=fpk_sb[:, mi : mi + 1],
                            scale=scal_sb[:, 0:1],
                        )
                        nc.vector.tensor_reduce(
                            maxcol[:, mi : mi + 1],
                            h_sb[:, mi, :],
                            axis=mybir.AxisListType.X,
                            op=ALU.max,
                            apply_absolute_value=True,
                        )

            # ---- phase 1.5: global scale via AllReduce(max) ------------------
            if p2:
                if not p1:
                    nc.vector.memset(maxcol[:], 1.0)
                    nc.vector.memset(h_sb[:], 0)
                pmax = consts.tile([128, 1], f32)
                nc.vector.tensor_reduce(
                    pmax[:], maxcol[:], axis=mybir.AxisListType.X, op=ALU.max
                )
                armax = consts.tile([128, 1], f32)
                if "no_collective" in flags:
                    nc.vector.tensor_copy(armax[:], pmax[:])
                else:
                    nc.gpsimd.dma_start(ar_in[:], pmax[:])
                    nc.gpsimd.collective_compute(
                        "AllReduce",
                        ALU.max,
                        replica_groups=[list(range(n_cores))],
                        ins=[ar_in[:]],
                        outs=[ar_out[:]],
                    )
                    nc.gpsimd.dma_start(armax[:], ar_out[:])
                gmax = consts.tile([128, 1], f32)
                if "no_par_reduce" in flags:
                    nc.vector.tensor_copy(gmax[:], armax[:])
                else:
                    nc.gpsimd.partition_all_reduce(
                        gmax[:], armax[:], channels=128, reduce_op=bass_isa.ReduceOp.max
                    )
                scaleh = consts.tile([128, 1], f32)
                invsh = consts.tile([128, 1], f32)
                s2v = consts.tile([128, 1], f32)
                c2v = consts.tile([128, 1], f32)
                nc.vector.tensor_scalar_max(gmax[:], gmax[:], 1e-8)
                # scale_h = gmax / 127  (mult by fp32(1/127): <=1ulp from divide)
                nc.vector.tensor_scalar_mul(
                    scaleh[:], gmax[:], float(np.float32(1.0) / np.float32(127.0))
                )
                nc.vector.reciprocal(invsh[:], scaleh[:])
                nc.vector.tensor_tensor(s2v[:], scaleh[:], scal_sb[:, 2:3], op=ALU.mult)
                nc.vector.reciprocal(c2v[:], s2v[:])
                nc.vector.tensor_scalar_mul(c2v[:], c2v[:], 2.0)

            # ---- phase 2: out = s2 * (qh@qW2 + lora2/s2) + b_proj ------------
            # xa2's 64 s_h-independent matmuls cross the AllReduce barrier on
            # the PE while qh production (ACT+DVE) waits for s_h.
            if not p2:
                pass
            else:
             with tc.tile_pool(name="qh", bufs=1) as qhp, tc.tile_pool(
                name="w2", bufs=3
            ) as w2p, tc.tile_pool(name="qt", bufs=3) as qtp, tc.tile_pool(
                name="ps2", bufs=2, space="PSUM"
            ) as pp2, tc.tile_pool(name="ot", bufs=4) as otp:
                qh_sb = qhp.tile([128, KO2, TC], bf16)
                ps_a2 = pp2.tile([128, TC], f32, tag="ps2_0", name="psa2")
                w2_first = []
                for ko in range(KO2):
                    if ko % W2CH == 0:
                        w2_sb = w2p.tile([128, W2CH, 512], bf16, tag="w2", name="w2c")
                        nc.sync.dma_start(w2_sb[:], blob_t[36 + ko // W2CH])
                        w2_first.append(w2_sb)
                    nc.tensor.matmul(
                        ps_a2[:R, :],
                        apk_sb[:, KO1 + ko, :],
                        h_sb[:, ko, :],
                        start=(ko == 0),
                        stop=(ko == KO2 - 1),
                    )
                    qt = qtp.tile([128, TC], f32, tag="qt", name="qt")
                    nc.scalar.activation(
                        qt[:], h_sb[:, ko, :], AF.Copy, bias=0.0, scale=invsh[:, 0:1]
                    )
                    nc.vector.tensor_scalar(
                        qh_sb[:, ko, :],
                        qt[:],
                        MAGIC,
                        MAGIC,
                        op0=ALU.add,
                        op1=ALU.subtract,
                    )
                nc.vector.tensor_scalar_mul(xa2_sb[:], ps_a2[:R, :], c2v[:R, 0:1])

                for no in range(NO2):
                    ps_list = [
                        pp2.tile([128, 512], f32, tag=f"ps2_{mi}", name="ps2t")
                        for mi in range(MT)
                    ]
                    for ko in range(KO2):
                        ch = ko // W2CH
                        if no == 0:
                            w2_sb = w2_first[ch]
                        elif ko % W2CH == 0:
                            w2_sb = w2p.tile(
                                [128, W2CH, 512], bf16, tag="w2", name="w2c"
                            )
                            nc.sync.dma_start(
                                w2_sb[:], blob_t[36 + 8 * no + ko // W2CH]
                            )
                        for mi in range(MT):
                            nc.tensor.matmul(
                                ps_list[mi][:],
                                qh_sb[:, ko, ts(mi, 128)],
                                w2_sb[:, ko % W2CH, :],
                                start=(ko == 0),
                                stop=False,
                            )
                    for mi in range(MT):
                        nc.tensor.matmul(
                            ps_list[mi][:],
                            xa2_sb[:, ts(mi, 128)],
                            bpj_sb[:, ds(no * 512, 512)],
                            start=False,
                            stop=True,
                        )
                        ot = otp.tile([128, 512], f32, tag="ot", name="ot")
                        # scale on ACT, bias-add on DVE (halves eviction latency
                        # at psum-bank reuse boundaries)
                        nc.scalar.activation(
                            ot[:], ps_list[mi][:], AF.Copy, bias=0.0, scale=s2v[:, 0:1]
                        )
                        nc.vector.tensor_add(
                            ot[:], ot[:], fpk_sb[:, ds(M64 + no * 512, 512)]
                        )
                        nc.scalar.dma_start(out_t[mi, :, ds(no * 512, 512)], ot[:])

    nc.compile()
    return nc


def _scale_of(a):
    m = np.max(np.abs(a)).astype(np.float32)
    m = np.maximum(m, np.float32(1e-8))
    return (m / QMAX).astype(np.float32)


def _quant(a, s):
    return np.clip(np.round(a / s), -QMAX, QMAX)


def _prep_weights(W_fc, b_fc, A_fc, B_fc, W_proj, b_proj, A_proj, B_proj):
    """Shared (replicated) input tensors from the weight arrays."""
    bf16 = ml_dtypes.bfloat16
    W_fc = np.asarray(W_fc, np.float32)
    W_proj = np.asarray(W_proj, np.float32)

    s_wfc = _scale_of(W_fc)
    s_wp = _scale_of(W_proj)
    qwfc = _quant(W_fc, s_wfc)
    qwp = _quant(W_proj, s_wp)

    # [k, ff] -> [mo, p(k%128), ko, c(ff%512)] -> chunks [2mo+half, p, 8, 512]
    wfc_dev = (
        qwfc.reshape(KO1, 128, MO1, 512)
        .transpose(2, 1, 0, 3)
        .astype(bf16)
        .reshape(MO1, 128, 2, W2CH, 512)
        .transpose(0, 2, 1, 3, 4)
        .reshape(2 * MO1, 128, W2CH, 512)
    )
    # [k, d] -> [no, p(k%128), ko, c(d%512)] -> chunks [8no+g, p, 8, 512]
    wpj_dev = (
        qwp.reshape(KO2, 128, NO2, 512)
        .transpose(2, 1, 0, 3)
        .astype(bf16)
        .reshape(NO2, 128, NCH2, W2CH, 512)
        .transpose(0, 2, 1, 3, 4)
        .reshape(NO2 * NCH2, 128, W2CH, 512)
    )
    wchunks = np.concatenate([wfc_dev, wpj_dev], axis=0)  # [64, 128, 8, 512]
    afc = np.asarray(A_fc, np.float32).reshape(KO1, 128, R).transpose(1, 0, 2)
    apj = np.asarray(A_proj, np.float32).reshape(KO2, 128, R).transpose(1, 0, 2)
    apk_dev = np.concatenate([afc, apj], axis=1).astype(bf16)  # [128, 80, 16]
    bpk_dev = np.concatenate(
        [np.asarray(B_fc, np.float32), np.asarray(B_proj, np.float32)], axis=1
    ).astype(bf16)  # [16, 10240]
    # chunks 68-71: A pack on chunk 68, B_fc on 69-70, B_proj on 71
    extra = np.zeros((4, 128, 4096), bf16)
    extra[0, :, 0:1280] = apk_dev.reshape(128, 1280)
    extra[1, 0:R, :] = bpk_dev[:, 0:4096]
    extra[2, 0:R, :] = bpk_dev[:, 4096:8192]
    extra[3, 0:R, 0:2048] = bpk_dev[:, 8192:10240]
    wchunks = np.ascontiguousarray(
        np.concatenate([wchunks, extra.reshape(4, 128, W2CH, 512)], axis=0)
    )  # [68, 128, 8, 512]
    fpk_dev = np.empty((128, M64 + D + 4), np.float32)
    fpk_dev[:, 0:M64] = np.asarray(b_fc, np.float32).reshape(M64, 128).T
    fpk_dev[:, M64 : M64 + D] = np.asarray(b_proj, np.float32)[None, :]
    return {
        "wchunks": wchunks,
        "fpk": fpk_dev,
        "s_wfc": s_wfc,
        "s_wp": s_wp,
    }


def _prep_inputs(hidden_states, W_fc, b_fc, A_fc, B_fc, W_proj, b_proj, A_proj, B_proj):
    bf16 = ml_dtypes.bfloat16
    w = _prep_weights(W_fc, b_fc, A_fc, B_fc, W_proj, b_proj, A_proj, B_proj)
    x = np.ascontiguousarray(np.asarray(hidden_states, np.float32).reshape(T, D))
    s_x = _scale_of(x)
    qx = _quant(x, s_x)
    s1 = np.float32(s_x * w["s_wfc"])
    c1 = np.float32(np.float32(2.0) / s1)
    fpk = np.zeros((128, W2CH * 512), np.float32)
    fpk[:, 0 : M64 + D + 4] = w["fpk"]
    fpk[:, M64 + D : M64 + D + 4] = np.array(
        [s1, c1, w["s_wp"], 0.0], np.float32
    )[None, :]
    # two-term bf16 split: hi + lo reconstructs fpk to ~2^-17 relative
    fpk_hi = fpk.astype(bf16)
    fpk_lo = (fpk - fpk_hi.astype(np.float32)).astype(bf16)
    fchunks = np.stack([fpk_hi, fpk_lo]).reshape(2, 128, W2CH, 512)

    in_maps = []
    for c in range(NCORES):
        xc = x[c * TC : (c + 1) * TC]  # [TC, D]
        qxc = qx[c * TC : (c + 1) * TC]
        qxT = qxc.T.reshape(KO1, 128, TC).transpose(1, 0, 2)
        xT = xc.T.reshape(KO1, 128, TC).transpose(1, 0, 2)
        # [128, 32ko, 512] -> 4 chunks [c, 128, 8, 512], then weight chunks
        xpk = (
            np.concatenate([qxT, xT], axis=1)
            .astype(bf16)
            .reshape(128, 4, W2CH, TC)
            .transpose(1, 0, 2, 3)
        )
        blob = np.ascontiguousarray(
            np.concatenate([xpk, w["wchunks"], fchunks], axis=0)
        )
        in_maps.append({"blob": blob})
    return in_maps


def _get_runner(**build_kwargs):
    """Build the Bass program once and wrap it in a cached jitted shard_map
    executable (adapted from concourse.bass2jax.run_bass_via_pjrt)."""
    key = ("runner", tuple(sorted(build_kwargs.items())))
    if key in _CACHE:
        return _CACHE[key]
    nc = _build_nc(**build_kwargs)
    n_cores_ = build_kwargs.get("n_cores", NCORES)
    runner = _runner_from_nc(nc, n_cores_)
    _CACHE[key] = runner
    return runner


def _runner_from_nc(nc, n_cores_):
    import jax
    import jax.numpy as jnp  # noqa: F401
    from jax.experimental.shard_map import shard_map
    from jax.sharding import Mesh, PartitionSpec

    from concourse import bass2jax, mybir

    bass2jax.install_neuronx_cc_hook()
    assert nc.dbg_addr is None
    partition_name = nc.partition_id_tensor.name if nc.partition_id_tensor else None

    in_names = []
    out_names = []
    out_avals = []
    for alloc in nc.m.functions[0].allocations:
        if not isinstance(alloc, mybir.MemoryLocationSet):
            continue
        name = alloc.memorylocations[0].name
        if alloc.kind == "ExternalInput":
            if name != partition_name:
                in_names.append(name)
        elif alloc.kind == "ExternalOutput":
            out_names.append(name)
            out_avals.append(
                jax.core.ShapedArray(tuple(alloc.tensor_shape), mybir.dt.np(alloc.dtype))
            )
    # Outputs are NOT passed as operands: the kernel writes every element of
    # its outputs, so the custom call's fresh (uninit) result buffers are
    # fully overwritten.  (The conventional donated-zero-buffer pattern only
    # exists for kernels that write outputs partially.)
    all_in_names = tuple(in_names)
    if partition_name is not None:
        all_in_names = all_in_names + (partition_name,)
    n_params = len(in_names)
    n_outs = len(out_names)

    def _body(*args):
        operands = list(args)
        if partition_name is not None:
            operands.append(bass2jax.partition_id_tensor())
        outs = bass2jax._bass_exec_p.bind(
            *operands,
            out_avals=tuple(out_avals),
            in_names=all_in_names,
            out_names=tuple(out_names),
            lowering_input_output_aliases=(),
            sim_require_finite=True,
            sim_require_nnan=True,
            nc=nc,
        )
        return tuple(outs)

    devices = jax.devices()[:n_cores_]
    assert len(devices) == n_cores_, f"need {n_cores_} devices, have {len(jax.devices())}"
    mesh = Mesh(np.asarray(devices), ("core",))
    in_specs = (PartitionSpec("core"),) * n_params
    out_specs = (PartitionSpec("core"),) * n_outs
    fn = jax.jit(
        shard_map(
            _body, mesh=mesh, in_specs=in_specs, out_specs=out_specs, check_rep=False
        ),
        keep_unused=True,
    )
    runner = {
        "fn": fn,
        "in_names": in_names,
        "out_names": out_names,
        "out_avals": out_avals,
        "mesh": mesh,
        "nc": nc,
    }
    runner["n_cores"] = n_cores_
    return runner


def _zero_outs(runner):
    n = runner["n_cores"]
    return [
        np.zeros((n * a.shape[0], *a.shape[1:]), a.dtype) for a in runner["out_avals"]
    ]


def _concat_inputs(in_maps, in_names):
    return [
        np.concatenate([m[name] for m in in_maps], axis=0) for name in in_names
    ]


def kernel(hidden_states, W_fc, b_fc, A_fc, B_fc, W_proj, b_proj, A_proj, B_proj):
    global LAST_RESULT
    runner = _get_runner()
    in_maps = _prep_inputs(
        hidden_states, W_fc, b_fc, A_fc, B_fc, W_proj, b_proj, A_proj, B_proj
    )
    concat_in = _concat_inputs(in_maps, runner["in_names"])
    out_arrs = runner["fn"](*concat_in)
    (out_global,) = [np.asarray(a) for a in out_arrs]
    # out_global: [NCORES*MT, 128, D] -> per-core [MT,128,D] -> tokens x D
    out = out_global.reshape(T, D).astype(np.float32)
    return out.reshape(B_, S, D)


def _dummy_in_maps():
    rng = np.random.default_rng(0)
    dummy = {
        "hidden_states": rng.standard_normal((B_, S, D), dtype=np.float32),
        "W_fc": rng.standard_normal((D, DFF), dtype=np.float32) / 45.0,
        "b_fc": np.zeros(DFF, np.float32),
        "A_fc": rng.standard_normal((D, R), dtype=np.float32) / 45.0,
        "B_fc": rng.standard_normal((R, DFF), dtype=np.float32) * 0.01,
        "W_proj": rng.standard_normal((DFF, D), dtype=np.float32) / 90.0,
        "b_proj": np.zeros(D, np.float32),
        "A_proj": rng.standard_normal((DFF, R), dtype=np.float32) / 90.0,
        "B_proj": rng.standard_normal((R, D), dtype=np.float32) * 0.01,
    }
    return _prep_inputs(**dummy)


def bench(n_iters=20, in_maps=None, rounds=5, **build_kwargs):
    """Per-iteration wall time of the full kernel (one dispatch per forward
    pass, device-resident inputs), best of ``rounds`` runs of ``n_iters``
    iterations each.  The axon tunnel's dispatch cost degrades one-sidedly
    over a session, so min-of-rounds is the robust steady-state estimate."""
    import time

    import jax
    from jax.sharding import NamedSharding, PartitionSpec

    runner = _get_runner(**build_kwargs)
    if in_maps is None:
        in_maps = _dummy_in_maps()
    concat_in = _concat_inputs(in_maps, runner["in_names"])
    sharding = NamedSharding(runner["mesh"], PartitionSpec("core"))
    dev_in = [jax.device_put(a, sharding) for a in concat_in]
    for a in dev_in:
        jax.block_until_ready(a)

    best = None
    for r in range(rounds):
        out = runner["fn"](*dev_in)  # warmup (compile on r=0)
        jax.block_until_ready(out)
        t0 = time.time()
        for i in range(n_iters):
            out = runner["fn"](*dev_in)
        jax.block_until_ready(out)
        dt = (time.time() - t0) / n_iters
        if best is None or dt < best:
            best = dt
    return best


def bench_device_loop(n_iters=20, in_maps=None, rounds=2, flags=()):
    """Device-side steady-state throughput: one dispatch executes a NEFF whose
    body is a hardware For loop running the forward pass ``n_iters`` times
    back-to-back, amortizing the per-call tunnel dispatch (~1.7ms on this
    setup).  The cross-core AllReduce desyncs the collective fabric when
    executed inside a hardware loop here, so this diagnostic runs the
    no_collective build (identical kernel minus the inter-core max exchange)."""
    import time

    import jax
    from jax.sharding import NamedSharding, PartitionSpec

    if "no_collective" not in flags:
        flags = ("no_collective",) + tuple(flags)
    runner = _get_runner(loop_k=n_iters, flags=flags)
    if in_maps is None:
        in_maps = _dummy_in_maps()
    concat_in = _concat_inputs(in_maps, runner["in_names"])
    sharding = NamedSharding(runner["mesh"], PartitionSpec("core"))
    dev_in = [jax.device_put(a, sharding) for a in concat_in]
    for a in dev_in:
        jax.block_until_ready(a)

    best = None
    for r in range(rounds + 1):  # first call = warmup (compile), not timed
        t0 = time.time()
        out = runner["fn"](*dev_in)
        jax.block_until_ready(out)
        dt = (time.time() - t0) / n_iters
        if r > 0 and (best is None or dt < best):
            best = dt
    return best
